# revision 40
# baseline (speedup 1.0000x reference)
"""AdaConv2d (per-sample masked 3x3 conv) on 8 TRN2 NeuronCores.

Strategy (data-parallel, per sharding hint):
  - 64 samples sharded 8-per-core; kernel_base/kernel_mask replicated
    (per-sample kernels base*mask[label] precomputed on host, bf16).
  - Host ships, per sample, a [128, 114*114] fp8-e3m4 buffer: BOTH
    partition halves hold the same zero-padded image (one input channel
    per partition).  e3m4 (4 mantissa bits) halves input DMA; measured
    rel err 1.36e-2 vs the 2e-2 budget.  The duplicate lets the two
    64-row halves of the PE array stream independent rhs data.
  - The PE array runs in 64x64 tiling mode: 4 independent tiles
    (SBUF half x PSUM half).  Each of the 9 conv taps is a K=64 matmul
    on one tile (moving fp8 x against stationary bf16 weights; the PE
    upconverts operands independently); per output block (4 rows x 112
    cols = 448 PSUM columns) the 9 taps split 4/5 between the two
    row-halves, accumulating into two PSUM banks (row tiles may not
    share a bank).  Column halves process the even/odd block of a
    block-pair.  All four tiles stream concurrently at 1 col/cycle each
    => 100% PE MAC utilization (the K=128 scheme wastes 25% on half-K
    passes).  Steady state measures at the MAC roofline (~12.2us per
    sample vs 11.8 ideal).
  - Eviction per block-pair: ACT copies the second PSUM bank to SBUF
    (f32), DVE adds it to the first bank with a bf16 cast; two pairs
    stage into one [128, 896] tile and DMA out with a plain 2D access
    pattern into a scratch DRAM layout (3D APs cost 5x on the DMA
    queue); the host un-shuffles to NCHW for free.
  - Queues: x chunks on gpsimd (no waits -> issues run ahead), weights
    + outputs on sync, evictions on scalar/vector.  Sample 0's chunks
    align to block-pair rows and its weight load is split per tap so
    the first matmuls start as soon as ~150KB lands.
"""
import numpy as np
import ml_dtypes

import concourse.bass as bass  # noqa: F401  (registers engines)
import concourse.tile as tile
from concourse import bacc, mybir
from concourse.bass_utils import run_bass_kernel_spmd

NCORES = 8
SPC = 8            # samples per core
H = W = 112
IC = OC = 64
ND = 4             # demographic groups
PW = H + 2         # padded width/height
PHW = PW * PW
RB = 4             # output rows per matmul block
N = RB * W         # 448 columns per matmul (one PSUM bank)
BLOCKS = H // RB   # 28 blocks per sample
NPAIRS = BLOCKS // 2
NT = 9             # taps
FUSE_EPOCH = 9
F32 = mybir.dt.float32
BF16 = mybir.dt.bfloat16
FP8E3 = mybir.dt.float8e3      # e3m4: 4 mantissa bits, range +-15.5

# tap splits per (pair+blk) parity; a chain on the top SBUF half pairs
# with the complementary chain on the bottom half so every tile does
# 4+5 taps per block-pair group of two
TAPS_A4 = [(0, 0), (0, 1), (0, 2), (2, 2)]
TAPS_B5 = [(1, 0), (1, 1), (1, 2), (2, 0), (2, 1)]
TAPS_A5 = [(0, 0), (0, 1), (0, 2), (2, 0), (2, 1)]
TAPS_B4 = [(1, 0), (1, 1), (1, 2), (2, 2)]

_CACHE = {}


def _emit_pair(nc, psX, psY, w3, x3, row_off, pair):
    """All 18 tap-matmuls of one block-pair, interleaved across the four
    PE tiles so their streams overlap."""
    chains = []
    for blk in range(2):
        b = 2 * pair + blk
        r0 = 4 * b - row_off
        pc = blk * 64
        if (pair + blk) % 2 == 0:
            h0, h1 = TAPS_A4, TAPS_B5
        else:
            h0, h1 = TAPS_A5, TAPS_B4
        chains.append((psX, pc, 0, h0, r0))
        chains.append((psY, pc, 64, h1, r0))
    for i in range(5):
        for ps, pc, sp, taps, r0 in chains:
            if i >= len(taps):
                continue
            dy, dx = taps[i]
            t = 3 * dy + dx
            nc.tensor.matmul(
                ps[pc:pc + 64, :],
                w3[sp:sp + 64, t, :],
                x3[sp:sp + 64, r0 + dy:r0 + dy + RB, dx:dx + W],
                start=(i == 0), stop=(i == len(taps) - 1))


def _build():
    nc = bacc.Bacc("TRN2", target_bir_lowering=False, debug=False,
                   num_devices=NCORES)
    # x ships as fp8 e3m4 (halves input DMA; rel err ~1.4e-2 vs 2e-2
    # budget).  The PE upconverts each operand independently, so the
    # moving fp8 stream pairs fine with bf16 stationary weights.
    xs = nc.dram_tensor("xs", [SPC, 128, PHW], FP8E3,
                        kind="ExternalInput").ap()
    # per-sample kernels precomputed on host (base*mask[label], bf16,
    # replicated on both partition halves)
    ws = nc.dram_tensor("ws", [SPC, 128, NT * OC], BF16,
                        kind="ExternalInput").ap()
    # scratch layout: [sample, pair-quad q, blk*64+oc, pr*448+rb*112+w]
    # (pair = 2q+pr).  Plain 2D [128, 896] DMAs; host un-shuffles to NCHW.
    out = nc.dram_tensor("out", [SPC, NPAIRS // 2, 128, 2 * N], BF16,
                         kind="ExternalOutput").ap()

    with tile.TileContext(nc) as tc:
        with (
            tc.tile_pool(name="xp", bufs=4) as xp,
            tc.tile_pool(name="wp", bufs=2) as wp,
            tc.tile_pool(name="tmp", bufs=4) as tp,
            tc.tile_pool(name="stage", bufs=8) as stp,
            tc.tile_pool(name="psumx", bufs=4, space="PSUM") as ppx,
            tc.tile_pool(name="psumy", bufs=4, space="PSUM") as ppy,
        ):
            for s in range(SPC):
                # wt first on sync so its transfer beats the x flood.
                # Sample 0's load is split per tap so the first LDWEIGHTS
                # only waits for one 16KB slice.
                wt = wp.tile([128, NT * OC], BF16, name="wt", tag="wt")
                if s == 0:
                    for t in range(NT):
                        nc.sync.dma_start(wt[:, t * OC:(t + 1) * OC],
                                          ws[s][:, t * OC:(t + 1) * OC])
                else:
                    nc.sync.dma_start(wt[:], ws[s])
                w3 = wt.rearrange("p (t oc) -> p t oc", oc=OC)

                # x chunks on gpsimd only: that queue carries no waits, so
                # issues flow freely ahead of compute.  Sample 0's chunks
                # align to block-pair rows (pair p needs padded rows
                # 8p..8p+9) so each pair starts as soon as its rows land.
                xt = xp.tile([128, PHW], FP8E3, name="xt", tag="xt")
                if s == 0:
                    # rows 0-3 land first (the very first matmul reads
                    # only those), then rows 4-9, then pair-aligned
                    bounds = [0, 4 * PW] \
                        + [(10 + 8 * k) * PW for k in range(13)] + [PHW]
                else:
                    NCH = 8
                    bounds = [(PHW // NCH) * q for q in range(NCH)] + [PHW]
                for qs, qe in zip(bounds, bounds[1:]):
                    nc.gpsimd.dma_start(xt[:, qs:qe], xs[s][:, qs:qe])
                x3 = xt.rearrange("p (r c) -> p r c", c=PW)

                for q in range(NPAIRS // 2):
                    # the run's very last quad writes per-pair so the
                    # final DMA doesn't wait for both evictions
                    last_quad = (s == SPC - 1 and q == NPAIRS // 2 - 1)
                    st = stp.tile([128, 2 * N], BF16, name="st", tag="st")
                    for pr in range(2):
                        pair = 2 * q + pr
                        psX = ppx.tile([128, N], F32, name="psX", tag="psX")
                        psY = ppy.tile([128, N], F32, name="psY", tag="psY")
                        _emit_pair(nc, psX, psY, w3, x3, 0, pair)

                        tmp = tp.tile([128, N], F32, name="tmp", tag="tmp")
                        nc.scalar.copy(tmp[:], psY[:])
                        nc.vector.tensor_tensor(st[:, pr * N:(pr + 1) * N],
                                                psX[:], tmp[:],
                                                op=mybir.AluOpType.add)
                        if last_quad:
                            nc.sync.dma_start(
                                out[s, q].rearrange(
                                    "p (pr n) -> p pr n", pr=2)[:, pr],
                                st[:, pr * N:(pr + 1) * N])
                    if not last_quad:
                        nc.sync.dma_start(out[s, q], st[:])

    nc.compile()
    return nc


def get_nc():
    if "nc" not in _CACHE:
        _CACHE["nc"] = _build()
    return _CACHE["nc"]


def make_in_maps(x, kernel_base, kernel_mask, demog_label, epoch):
    kb = np.asarray(kernel_base, dtype=np.float32)
    km = np.asarray(kernel_mask, dtype=np.float32)
    labels = np.asarray(demog_label).astype(np.int64)
    if int(np.asarray(epoch)) >= FUSE_EPOCH:
        labels = np.zeros_like(labels)

    B = labels.shape[0]
    # padded fp8(e3m4) image duplicated on both partition halves
    xb = np.asarray(x, dtype=np.float32).astype(ml_dtypes.float8_e3m4)
    xpad = np.zeros((B, IC, PW, PW), dtype=ml_dtypes.float8_e3m4)
    xpad[:, :, 1:H + 1, 1:W + 1] = xb
    flat = xpad.reshape(B, IC, PHW)
    xfull = np.empty((B, 128, PHW), dtype=ml_dtypes.float8_e3m4)
    xfull[:, 0:IC, :] = flat
    xfull[:, IC:, :] = flat

    # per-sample kernels: ws[b, p, t*64+oc] = (kb*km[label])[oc, p%64, t]
    kb9 = kb.reshape(OC, IC, NT)           # tap index = 3*dy + dx
    km9 = km.reshape(ND, IC, NT)
    wk = kb9[None] * km9[labels][:, None]              # [B, oc, ic, t]
    wsall = np.empty((B, 128, NT, OC), dtype=ml_dtypes.bfloat16)
    wsall[:, 0:IC] = wk.transpose(0, 2, 3, 1)          # [B, ic, t, oc]
    wsall[:, IC:] = wsall[:, 0:IC]
    wsall = wsall.reshape(B, 128, NT * OC)

    in_maps = []
    for c in range(NCORES):
        in_maps.append({
            "xs": np.ascontiguousarray(xfull[c * SPC:(c + 1) * SPC]),
            "ws": np.ascontiguousarray(wsall[c * SPC:(c + 1) * SPC]),
        })
    return in_maps


def kernel(x, kernel_base, kernel_mask, demog_label, epoch):
    nc = get_nc()
    in_maps = make_in_maps(x, kernel_base, kernel_mask, demog_label, epoch)
    res = run_bass_kernel_spmd(nc, in_maps, list(range(NCORES)))
    outs = []
    for c in range(NCORES):
        # [s, q, blk*64+oc, pr*448+rb*112+w] -> [s, oc, h, w],
        # h = q*16 + pr*8 + blk*4 + rb
        arr = res.results[c]["out"].astype(np.float32)
        arr = arr.reshape(SPC, NPAIRS // 2, 2, OC, 2, RB, W)
        arr = arr.transpose(0, 3, 1, 4, 2, 5, 6).reshape(SPC, OC, H, W)
        outs.append(arr)
    return np.concatenate(outs, axis=0)


# revision 42
# speedup vs baseline: 1.0157x; 1.0157x over previous
"""AdaConv2d (per-sample masked 3x3 conv) on 8 TRN2 NeuronCores.

Strategy (data-parallel, per sharding hint):
  - 64 samples sharded 8-per-core; kernel_base/kernel_mask replicated
    (per-sample kernels base*mask[label] precomputed on host, bf16).
  - Host ships, per sample, a [128, 114*114] fp8-e3m4 buffer: BOTH
    partition halves hold the same zero-padded image (one input channel
    per partition).  e3m4 (4 mantissa bits) halves input DMA; measured
    rel err 1.36e-2 vs the 2e-2 budget.  The duplicate lets the two
    64-row halves of the PE array stream independent rhs data.
  - The PE array runs in 64x64 tiling mode: 4 independent tiles
    (SBUF half x PSUM half).  Each of the 9 conv taps is a K=64 matmul
    on one tile (moving fp8 x against stationary bf16 weights; the PE
    upconverts operands independently); per output block (4 rows x 112
    cols = 448 PSUM columns) the 9 taps split 4/5 between the two
    row-halves, accumulating into two PSUM banks (row tiles may not
    share a bank).  Column halves process the even/odd block of a
    block-pair.  All four tiles stream concurrently at 1 col/cycle each
    => 100% PE MAC utilization (the K=128 scheme wastes 25% on half-K
    passes).  Steady state measures at the MAC roofline (~12.2us per
    sample vs 11.8 ideal).
  - Eviction per block-pair: ACT copies the second PSUM bank to SBUF
    (f32), DVE adds it to the first bank with a bf16 cast; two pairs
    stage into one [128, 896] tile and DMA out with a plain 2D access
    pattern into a scratch DRAM layout (3D APs cost 5x on the DMA
    queue); the host un-shuffles to NCHW for free.
  - Queues: x chunks on gpsimd (no waits -> issues run ahead), weights
    + outputs on sync, evictions on scalar/vector.  Sample 0's chunks
    align to block-pair rows and its weight load is split per tap so
    the first matmuls start as soon as ~150KB lands.
"""
import numpy as np
import ml_dtypes

import concourse.bass as bass  # noqa: F401  (registers engines)
import concourse.tile as tile
from concourse import bacc, mybir
from concourse.bass_utils import run_bass_kernel_spmd

NCORES = 8
SPC = 8            # samples per core
H = W = 112
IC = OC = 64
ND = 4             # demographic groups
PW = H + 2         # padded width/height
PHW = PW * PW
RB = 4             # output rows per matmul block
N = RB * W         # 448 columns per matmul (one PSUM bank)
BLOCKS = H // RB   # 28 blocks per sample
NPAIRS = BLOCKS // 2
NT = 9             # taps
FUSE_EPOCH = 9
F32 = mybir.dt.float32
BF16 = mybir.dt.bfloat16
FP8E3 = mybir.dt.float8e3      # e3m4: 4 mantissa bits, range +-15.5

# tap splits per (pair+blk) parity; a chain on the top SBUF half pairs
# with the complementary chain on the bottom half so every tile does
# 4+5 taps per block-pair group of two
TAPS_A4 = [(0, 0), (0, 1), (0, 2), (2, 2)]
TAPS_B5 = [(1, 0), (1, 1), (1, 2), (2, 0), (2, 1)]
TAPS_A5 = [(0, 0), (0, 1), (0, 2), (2, 0), (2, 1)]
TAPS_B4 = [(1, 0), (1, 1), (1, 2), (2, 2)]

_CACHE = {}


def _emit_pair(nc, psX, psY, w3, x3, row_off, pair):
    """All 18 tap-matmuls of one block-pair, interleaved across the four
    PE tiles so their streams overlap."""
    chains = []
    for blk in range(2):
        b = 2 * pair + blk
        r0 = 4 * b - row_off
        pc = blk * 64
        if (pair + blk) % 2 == 0:
            h0, h1 = TAPS_A4, TAPS_B5
        else:
            h0, h1 = TAPS_A5, TAPS_B4
        chains.append((psX, pc, 0, h0, r0))
        chains.append((psY, pc, 64, h1, r0))
    for i in range(5):
        for ps, pc, sp, taps, r0 in chains:
            if i >= len(taps):
                continue
            dy, dx = taps[i]
            t = 3 * dy + dx
            nc.tensor.matmul(
                ps[pc:pc + 64, :],
                w3[sp:sp + 64, t, :],
                x3[sp:sp + 64, r0 + dy:r0 + dy + RB, dx:dx + W],
                start=(i == 0), stop=(i == len(taps) - 1))


def _build():
    nc = bacc.Bacc("TRN2", target_bir_lowering=False, debug=False,
                   num_devices=NCORES)
    # x ships as fp8 e3m4 (halves input DMA; rel err ~1.4e-2 vs 2e-2
    # budget).  The PE upconverts each operand independently, so the
    # moving fp8 stream pairs fine with bf16 stationary weights.
    xs = nc.dram_tensor("xs", [SPC, 128, PHW], FP8E3,
                        kind="ExternalInput").ap()
    # per-sample kernels precomputed on host (base*mask[label], bf16,
    # replicated on both partition halves)
    ws = nc.dram_tensor("ws", [SPC, 128, NT * OC], BF16,
                        kind="ExternalInput").ap()
    # scratch layout: [sample, pair-quad q, blk*64+oc, pr*448+rb*112+w]
    # (pair = 2q+pr).  Plain 2D [128, 896] DMAs; host un-shuffles to NCHW.
    out = nc.dram_tensor("out", [SPC, NPAIRS // 2, 128, 2 * N], BF16,
                         kind="ExternalOutput").ap()

    with tile.TileContext(nc) as tc:
        with (
            tc.tile_pool(name="xp", bufs=3) as xp,
            tc.tile_pool(name="wp", bufs=2) as wp,
            tc.tile_pool(name="tmp", bufs=4) as tp,
            tc.tile_pool(name="stage", bufs=8) as stp,
            tc.tile_pool(name="psumx", bufs=4, space="PSUM") as ppx,
            tc.tile_pool(name="psumy", bufs=4, space="PSUM") as ppy,
        ):
            for s in range(SPC):
                # wt first on sync so its transfer beats the x flood.
                # Sample 0's load is split per tap so the first LDWEIGHTS
                # only waits for one 16KB slice.
                wt = wp.tile([128, NT * OC], BF16, name="wt", tag="wt")
                if s == 0:
                    for t in range(NT):
                        nc.sync.dma_start(wt[:, t * OC:(t + 1) * OC],
                                          ws[s][:, t * OC:(t + 1) * OC])
                else:
                    nc.sync.dma_start(wt[:], ws[s])
                w3 = wt.rearrange("p (t oc) -> p t oc", oc=OC)

                # x chunks on gpsimd only: that queue carries no waits, so
                # issues flow freely ahead of compute.  Sample 0's chunks
                # align to block-pair rows (pair p needs padded rows
                # 8p..8p+9) so each pair starts as soon as its rows land.
                xt = xp.tile([128, PHW], FP8E3, name="xt", tag="xt")
                if s == 0:
                    # rows 0-3 land first (the very first matmul reads
                    # only those), then rows 4-9, then pair-aligned
                    bounds = [0, 4 * PW] \
                        + [(10 + 8 * k) * PW for k in range(13)] + [PHW]
                else:
                    NCH = 8
                    bounds = [(PHW // NCH) * q for q in range(NCH)] + [PHW]
                for qs, qe in zip(bounds, bounds[1:]):
                    nc.gpsimd.dma_start(xt[:, qs:qe], xs[s][:, qs:qe])
                x3 = xt.rearrange("p (r c) -> p r c", c=PW)

                for q in range(NPAIRS // 2):
                    st = stp.tile([128, 2 * N], BF16, name="st", tag="st")
                    for pr in range(2):
                        pair = 2 * q + pr
                        psX = ppx.tile([128, N], F32, name="psX", tag="psX")
                        psY = ppy.tile([128, N], F32, name="psY", tag="psY")
                        _emit_pair(nc, psX, psY, w3, x3, 0, pair)

                        tmp = tp.tile([128, N], F32, name="tmp", tag="tmp")
                        nc.scalar.copy(tmp[:], psY[:])
                        nc.vector.tensor_tensor(st[:, pr * N:(pr + 1) * N],
                                                psX[:], tmp[:],
                                                op=mybir.AluOpType.add)
                    nc.sync.dma_start(out[s, q], st[:])

    nc.compile()
    return nc


def get_nc():
    if "nc" not in _CACHE:
        _CACHE["nc"] = _build()
    return _CACHE["nc"]


def make_in_maps(x, kernel_base, kernel_mask, demog_label, epoch):
    kb = np.asarray(kernel_base, dtype=np.float32)
    km = np.asarray(kernel_mask, dtype=np.float32)
    labels = np.asarray(demog_label).astype(np.int64)
    if int(np.asarray(epoch)) >= FUSE_EPOCH:
        labels = np.zeros_like(labels)

    B = labels.shape[0]
    # padded fp8(e3m4) image duplicated on both partition halves
    xb = np.asarray(x, dtype=np.float32).astype(ml_dtypes.float8_e3m4)
    xpad = np.zeros((B, IC, PW, PW), dtype=ml_dtypes.float8_e3m4)
    xpad[:, :, 1:H + 1, 1:W + 1] = xb
    flat = xpad.reshape(B, IC, PHW)
    xfull = np.empty((B, 128, PHW), dtype=ml_dtypes.float8_e3m4)
    xfull[:, 0:IC, :] = flat
    xfull[:, IC:, :] = flat

    # per-sample kernels: ws[b, p, t*64+oc] = (kb*km[label])[oc, p%64, t]
    kb9 = kb.reshape(OC, IC, NT)           # tap index = 3*dy + dx
    km9 = km.reshape(ND, IC, NT)
    wk = kb9[None] * km9[labels][:, None]              # [B, oc, ic, t]
    wsall = np.empty((B, 128, NT, OC), dtype=ml_dtypes.bfloat16)
    wsall[:, 0:IC] = wk.transpose(0, 2, 3, 1)          # [B, ic, t, oc]
    wsall[:, IC:] = wsall[:, 0:IC]
    wsall = wsall.reshape(B, 128, NT * OC)

    in_maps = []
    for c in range(NCORES):
        in_maps.append({
            "xs": np.ascontiguousarray(xfull[c * SPC:(c + 1) * SPC]),
            "ws": np.ascontiguousarray(wsall[c * SPC:(c + 1) * SPC]),
        })
    return in_maps


def kernel(x, kernel_base, kernel_mask, demog_label, epoch):
    nc = get_nc()
    in_maps = make_in_maps(x, kernel_base, kernel_mask, demog_label, epoch)
    res = run_bass_kernel_spmd(nc, in_maps, list(range(NCORES)))
    outs = []
    for c in range(NCORES):
        # [s, q, blk*64+oc, pr*448+rb*112+w] -> [s, oc, h, w],
        # h = q*16 + pr*8 + blk*4 + rb
        arr = res.results[c]["out"].astype(np.float32)
        arr = arr.reshape(SPC, NPAIRS // 2, 2, OC, 2, RB, W)
        arr = arr.transpose(0, 3, 1, 4, 2, 5, 6).reshape(SPC, OC, H, W)
        outs.append(arr)
    return np.concatenate(outs, axis=0)


# revision 44
# speedup vs baseline: 1.0321x; 1.0161x over previous
"""AdaConv2d (per-sample masked 3x3 conv) on 8 TRN2 NeuronCores.

Strategy (data-parallel, per sharding hint):
  - 64 samples sharded 8-per-core; kernel_base/kernel_mask replicated
    (per-sample kernels base*mask[label] precomputed on host, bf16).
  - Host ships, per sample, a [128, 114*114] fp8-e3m4 buffer: BOTH
    partition halves hold the same zero-padded image (one input channel
    per partition).  e3m4 (4 mantissa bits) halves input DMA; measured
    rel err 1.36e-2 vs the 2e-2 budget.  The duplicate lets the two
    64-row halves of the PE array stream independent rhs data.
  - The PE array runs in 64x64 tiling mode: 4 independent tiles
    (SBUF half x PSUM half).  Each of the 9 conv taps is a K=64 matmul
    on one tile (moving fp8 x against stationary bf16 weights; the PE
    upconverts operands independently); per output block (4 rows x 112
    cols = 448 PSUM columns) the 9 taps split 4/5 between the two
    row-halves, accumulating into two PSUM banks (row tiles may not
    share a bank).  Column halves process the even/odd block of a
    block-pair.  All four tiles stream concurrently at 1 col/cycle each
    => 100% PE MAC utilization (the K=128 scheme wastes 25% on half-K
    passes).  Steady state measures at the MAC roofline (~12.2us per
    sample vs 11.8 ideal).
  - Eviction per block-pair: ACT copies the second PSUM bank to SBUF
    (f32), DVE adds it to the first bank with a bf16 cast; two pairs
    stage into one [128, 896] tile and DMA out with a plain 2D access
    pattern into a scratch DRAM layout (3D APs cost 5x on the DMA
    queue); the host un-shuffles to NCHW for free.
  - Queues: x chunks on gpsimd (no waits -> issues run ahead), weights
    + outputs on sync, evictions on scalar/vector.  Sample 0's chunks
    align to block-pair rows and its weight load is split per tap so
    the first matmuls start as soon as ~150KB lands.
"""
import numpy as np
import ml_dtypes

import concourse.bass as bass  # noqa: F401  (registers engines)
import concourse.tile as tile
from concourse import bacc, mybir
from concourse.bass_utils import run_bass_kernel_spmd

NCORES = 8
SPC = 8            # samples per core
H = W = 112
IC = OC = 64
ND = 4             # demographic groups
PW = H + 2         # padded width/height
PHW = PW * PW
RB = 4             # output rows per matmul block
N = RB * W         # 448 columns per matmul (one PSUM bank)
BLOCKS = H // RB   # 28 blocks per sample
NPAIRS = BLOCKS // 2
NT = 9             # taps
FUSE_EPOCH = 9
F32 = mybir.dt.float32
BF16 = mybir.dt.bfloat16
FP8E3 = mybir.dt.float8e3      # e3m4: 4 mantissa bits, range +-15.5

# tap splits per (pair+blk) parity; a chain on the top SBUF half pairs
# with the complementary chain on the bottom half so every tile does
# 4+5 taps per block-pair group of two
TAPS_A4 = [(0, 0), (0, 1), (0, 2), (2, 2)]
TAPS_B5 = [(1, 0), (1, 1), (1, 2), (2, 0), (2, 1)]
TAPS_A5 = [(0, 0), (0, 1), (0, 2), (2, 0), (2, 1)]
TAPS_B4 = [(1, 0), (1, 1), (1, 2), (2, 2)]

_CACHE = {}


def _emit_pair(nc, psX, psY, w3, x3, row_off, pair):
    """All 18 tap-matmuls of one block-pair, interleaved across the four
    PE tiles so their streams overlap."""
    chains = []
    for blk in range(2):
        b = 2 * pair + blk
        r0 = 4 * b - row_off
        pc = blk * 64
        if (pair + blk) % 2 == 0:
            h0, h1 = TAPS_A4, TAPS_B5
        else:
            h0, h1 = TAPS_A5, TAPS_B4
        chains.append((psX, pc, 0, h0, r0))
        chains.append((psY, pc, 64, h1, r0))
    for i in range(5):
        for ps, pc, sp, taps, r0 in chains:
            if i >= len(taps):
                continue
            dy, dx = taps[i]
            t = 3 * dy + dx
            nc.tensor.matmul(
                ps[pc:pc + 64, :],
                w3[sp:sp + 64, t, :],
                x3[sp:sp + 64, r0 + dy:r0 + dy + RB, dx:dx + W],
                start=(i == 0), stop=(i == len(taps) - 1))


def _build():
    nc = bacc.Bacc("TRN2", target_bir_lowering=False, debug=False,
                   num_devices=NCORES)
    # x ships as fp8 e3m4 (halves input DMA; rel err ~1.4e-2 vs 2e-2
    # budget).  The PE upconverts each operand independently, so the
    # moving fp8 stream pairs fine with bf16 stationary weights.
    xs = nc.dram_tensor("xs", [SPC, 128, PHW], FP8E3,
                        kind="ExternalInput").ap()
    # per-sample kernels precomputed on host (base*mask[label], bf16,
    # replicated on both partition halves)
    ws = nc.dram_tensor("ws", [SPC, 128, NT * OC], BF16,
                        kind="ExternalInput").ap()
    # scratch layout: [sample, pair-quad q, blk*64+oc, pr*448+rb*112+w]
    # (pair = 2q+pr).  Plain 2D [128, 896] DMAs; host un-shuffles to NCHW.
    out = nc.dram_tensor("out", [SPC, NPAIRS // 2, 128, 2 * N], BF16,
                         kind="ExternalOutput").ap()

    with tile.TileContext(nc) as tc:
        with (
            tc.tile_pool(name="xp", bufs=3) as xp,
            tc.tile_pool(name="wp", bufs=2) as wp,
            tc.tile_pool(name="tmp", bufs=4) as tp,
            tc.tile_pool(name="stage", bufs=8) as stp,
            tc.tile_pool(name="psumx", bufs=4, space="PSUM") as ppx,
            tc.tile_pool(name="psumy", bufs=4, space="PSUM") as ppy,
        ):
            for s in range(SPC):
                # wt first on sync so its transfer beats the x flood.
                # Sample 0's load is split per tap so the first LDWEIGHTS
                # only waits for one 16KB slice.
                wt = wp.tile([128, NT * OC], BF16, name="wt", tag="wt")
                if s == 0:
                    for t in range(NT):
                        nc.sync.dma_start(wt[:, t * OC:(t + 1) * OC],
                                          ws[s][:, t * OC:(t + 1) * OC])
                else:
                    nc.sync.dma_start(wt[:], ws[s])
                w3 = wt.rearrange("p (t oc) -> p t oc", oc=OC)

                # x chunks on gpsimd only: that queue carries no waits, so
                # issues flow freely ahead of compute.  Sample 0's chunks
                # align to block-pair rows (pair p needs padded rows
                # 8p..8p+9) so each pair starts as soon as its rows land.
                xt = xp.tile([128, PHW], FP8E3, name="xt", tag="xt")
                if s == 0:
                    bounds = [0] + [(10 + 8 * k) * PW for k in range(13)] \
                        + [PHW]
                else:
                    # coarse chunks: fewer issues drain the gpsimd queue
                    # faster so later samples' prefetch starts earlier
                    NCH = 4
                    bounds = [(PHW // NCH) * q for q in range(NCH)] + [PHW]
                for qs, qe in zip(bounds, bounds[1:]):
                    nc.gpsimd.dma_start(xt[:, qs:qe], xs[s][:, qs:qe])
                x3 = xt.rearrange("p (r c) -> p r c", c=PW)

                for q in range(NPAIRS // 2):
                    st = stp.tile([128, 2 * N], BF16, name="st", tag="st")
                    for pr in range(2):
                        pair = 2 * q + pr
                        psX = ppx.tile([128, N], F32, name="psX", tag="psX")
                        psY = ppy.tile([128, N], F32, name="psY", tag="psY")
                        _emit_pair(nc, psX, psY, w3, x3, 0, pair)

                        tmp = tp.tile([128, N], F32, name="tmp", tag="tmp")
                        nc.scalar.copy(tmp[:], psY[:])
                        nc.vector.tensor_tensor(st[:, pr * N:(pr + 1) * N],
                                                psX[:], tmp[:],
                                                op=mybir.AluOpType.add)
                    nc.sync.dma_start(out[s, q], st[:])

    nc.compile()
    return nc


def get_nc():
    if "nc" not in _CACHE:
        _CACHE["nc"] = _build()
    return _CACHE["nc"]


def make_in_maps(x, kernel_base, kernel_mask, demog_label, epoch):
    kb = np.asarray(kernel_base, dtype=np.float32)
    km = np.asarray(kernel_mask, dtype=np.float32)
    labels = np.asarray(demog_label).astype(np.int64)
    if int(np.asarray(epoch)) >= FUSE_EPOCH:
        labels = np.zeros_like(labels)

    B = labels.shape[0]
    # padded fp8(e3m4) image duplicated on both partition halves
    xb = np.asarray(x, dtype=np.float32).astype(ml_dtypes.float8_e3m4)
    xpad = np.zeros((B, IC, PW, PW), dtype=ml_dtypes.float8_e3m4)
    xpad[:, :, 1:H + 1, 1:W + 1] = xb
    flat = xpad.reshape(B, IC, PHW)
    xfull = np.empty((B, 128, PHW), dtype=ml_dtypes.float8_e3m4)
    xfull[:, 0:IC, :] = flat
    xfull[:, IC:, :] = flat

    # per-sample kernels: ws[b, p, t*64+oc] = (kb*km[label])[oc, p%64, t]
    kb9 = kb.reshape(OC, IC, NT)           # tap index = 3*dy + dx
    km9 = km.reshape(ND, IC, NT)
    wk = kb9[None] * km9[labels][:, None]              # [B, oc, ic, t]
    wsall = np.empty((B, 128, NT, OC), dtype=ml_dtypes.bfloat16)
    wsall[:, 0:IC] = wk.transpose(0, 2, 3, 1)          # [B, ic, t, oc]
    wsall[:, IC:] = wsall[:, 0:IC]
    wsall = wsall.reshape(B, 128, NT * OC)

    in_maps = []
    for c in range(NCORES):
        in_maps.append({
            "xs": np.ascontiguousarray(xfull[c * SPC:(c + 1) * SPC]),
            "ws": np.ascontiguousarray(wsall[c * SPC:(c + 1) * SPC]),
        })
    return in_maps


def kernel(x, kernel_base, kernel_mask, demog_label, epoch):
    nc = get_nc()
    in_maps = make_in_maps(x, kernel_base, kernel_mask, demog_label, epoch)
    res = run_bass_kernel_spmd(nc, in_maps, list(range(NCORES)))
    outs = []
    for c in range(NCORES):
        # [s, q, blk*64+oc, pr*448+rb*112+w] -> [s, oc, h, w],
        # h = q*16 + pr*8 + blk*4 + rb
        arr = res.results[c]["out"].astype(np.float32)
        arr = arr.reshape(SPC, NPAIRS // 2, 2, OC, 2, RB, W)
        arr = arr.transpose(0, 3, 1, 4, 2, 5, 6).reshape(SPC, OC, H, W)
        outs.append(arr)
    return np.concatenate(outs, axis=0)


# revision 45
# speedup vs baseline: 1.0383x; 1.0060x over previous
"""AdaConv2d (per-sample masked 3x3 conv) on 8 TRN2 NeuronCores.

Strategy (data-parallel, per sharding hint):
  - 64 samples sharded 8-per-core; kernel_base/kernel_mask replicated
    (per-sample kernels base*mask[label] precomputed on host, bf16).
  - Host ships, per sample, a [128, 114*114] fp8-e3m4 buffer: BOTH
    partition halves hold the same zero-padded image (one input channel
    per partition).  e3m4 (4 mantissa bits) halves input DMA; measured
    rel err 1.36e-2 vs the 2e-2 budget.  The duplicate lets the two
    64-row halves of the PE array stream independent rhs data.
  - The PE array runs in 64x64 tiling mode: 4 independent tiles
    (SBUF half x PSUM half).  Each of the 9 conv taps is a K=64 matmul
    on one tile (moving fp8 x against stationary bf16 weights; the PE
    upconverts operands independently); per output block (4 rows x 112
    cols = 448 PSUM columns) the 9 taps split 4/5 between the two
    row-halves, accumulating into two PSUM banks (row tiles may not
    share a bank).  Column halves process the even/odd block of a
    block-pair.  All four tiles stream concurrently at 1 col/cycle each
    => 100% PE MAC utilization (the K=128 scheme wastes 25% on half-K
    passes).  Steady state measures at the MAC roofline (~12.2us per
    sample vs 11.8 ideal).
  - Eviction per block-pair: ACT copies the second PSUM bank to SBUF
    (f32), DVE adds it to the first bank with a bf16 cast; two pairs
    stage into one [128, 896] tile and DMA out with a plain 2D access
    pattern into a scratch DRAM layout (3D APs cost 5x on the DMA
    queue); the host un-shuffles to NCHW for free.
  - Queues: x chunks on gpsimd (no waits -> issues run ahead), weights
    + outputs on sync, evictions on scalar/vector.  Sample 0's chunks
    align to block-pair rows and its weight load is split per tap so
    the first matmuls start as soon as ~150KB lands.
"""
import numpy as np
import ml_dtypes

import concourse.bass as bass  # noqa: F401  (registers engines)
import concourse.tile as tile
from concourse import bacc, mybir
from concourse.bass_utils import run_bass_kernel_spmd

NCORES = 8
SPC = 8            # samples per core
H = W = 112
IC = OC = 64
ND = 4             # demographic groups
PW = H + 2         # padded width/height
PHW = PW * PW
RB = 4             # output rows per matmul block
N = RB * W         # 448 columns per matmul (one PSUM bank)
BLOCKS = H // RB   # 28 blocks per sample
NPAIRS = BLOCKS // 2
NT = 9             # taps
FUSE_EPOCH = 9
F32 = mybir.dt.float32
BF16 = mybir.dt.bfloat16
FP8E3 = mybir.dt.float8e3      # e3m4: 4 mantissa bits, range +-15.5

# tap splits per (pair+blk) parity; a chain on the top SBUF half pairs
# with the complementary chain on the bottom half so every tile does
# 4+5 taps per block-pair group of two
TAPS_A4 = [(0, 0), (0, 1), (0, 2), (2, 2)]
TAPS_B5 = [(1, 0), (1, 1), (1, 2), (2, 0), (2, 1)]
TAPS_A5 = [(0, 0), (0, 1), (0, 2), (2, 0), (2, 1)]
TAPS_B4 = [(1, 0), (1, 1), (1, 2), (2, 2)]

_CACHE = {}


def _emit_pair(nc, psX, psY, w3, x3, row_off, pair):
    """All 18 tap-matmuls of one block-pair, interleaved across the four
    PE tiles so their streams overlap."""
    chains = []
    for blk in range(2):
        b = 2 * pair + blk
        r0 = 4 * b - row_off
        pc = blk * 64
        if (pair + blk) % 2 == 0:
            h0, h1 = TAPS_A4, TAPS_B5
        else:
            h0, h1 = TAPS_A5, TAPS_B4
        chains.append((psX, pc, 0, h0, r0))
        chains.append((psY, pc, 64, h1, r0))
    for i in range(5):
        for ps, pc, sp, taps, r0 in chains:
            if i >= len(taps):
                continue
            dy, dx = taps[i]
            t = 3 * dy + dx
            nc.tensor.matmul(
                ps[pc:pc + 64, :],
                w3[sp:sp + 64, t, :],
                x3[sp:sp + 64, r0 + dy:r0 + dy + RB, dx:dx + W],
                start=(i == 0), stop=(i == len(taps) - 1))


def _build():
    nc = bacc.Bacc("TRN2", target_bir_lowering=False, debug=False,
                   num_devices=NCORES)
    # x ships as fp8 e3m4 (halves input DMA; rel err ~1.4e-2 vs 2e-2
    # budget).  The PE upconverts each operand independently, so the
    # moving fp8 stream pairs fine with bf16 stationary weights.
    xs = nc.dram_tensor("xs", [SPC, 128, PHW], FP8E3,
                        kind="ExternalInput").ap()
    # per-sample kernels precomputed on host (base*mask[label], bf16,
    # replicated on both partition halves)
    ws = nc.dram_tensor("ws", [SPC, 128, NT * OC], BF16,
                        kind="ExternalInput").ap()
    # scratch layout: [sample, pair-quad q, blk*64+oc, pr*448+rb*112+w]
    # (pair = 2q+pr).  Plain 2D [128, 896] DMAs; host un-shuffles to NCHW.
    out = nc.dram_tensor("out", [SPC, NPAIRS // 2, 128, 2 * N], BF16,
                         kind="ExternalOutput").ap()

    with tile.TileContext(nc) as tc:
        with (
            tc.tile_pool(name="xp", bufs=3) as xp,
            tc.tile_pool(name="wp", bufs=2) as wp,
            tc.tile_pool(name="tmp", bufs=4) as tp,
            tc.tile_pool(name="stage", bufs=8) as stp,
            tc.tile_pool(name="psumx", bufs=4, space="PSUM") as ppx,
            tc.tile_pool(name="psumy", bufs=4, space="PSUM") as ppy,
        ):
            for s in range(SPC):
                # wt first on sync so its transfer beats the x flood.
                # Sample 0's load is split per tap so the first LDWEIGHTS
                # only waits for one 16KB slice.
                wt = wp.tile([128, NT * OC], BF16, name="wt", tag="wt")
                if s == 0:
                    for t in range(NT):
                        nc.sync.dma_start(wt[:, t * OC:(t + 1) * OC],
                                          ws[s][:, t * OC:(t + 1) * OC])
                else:
                    nc.sync.dma_start(wt[:], ws[s])
                w3 = wt.rearrange("p (t oc) -> p t oc", oc=OC)

                # x chunks on gpsimd only: that queue carries no waits, so
                # issues flow freely ahead of compute.  Sample 0's chunks
                # align to block-pair rows (pair p needs padded rows
                # 8p..8p+9) so each pair starts as soon as its rows land.
                xt = xp.tile([128, PHW], FP8E3, name="xt", tag="xt")
                if s == 0:
                    bounds = [0] + [(10 + 8 * k) * PW for k in range(13)] \
                        + [PHW]
                else:
                    NCH = 8
                    bounds = [(PHW // NCH) * q for q in range(NCH)] + [PHW]
                for qs, qe in zip(bounds, bounds[1:]):
                    nc.gpsimd.dma_start(xt[:, qs:qe], xs[s][:, qs:qe])
                x3 = xt.rearrange("p (r c) -> p r c", c=PW)

                for q in range(NPAIRS // 2):
                    st = stp.tile([128, 2 * N], BF16, name="st", tag="st")
                    for pr in range(2):
                        pair = 2 * q + pr
                        psX = ppx.tile([128, N], F32, name="psX", tag="psX")
                        psY = ppy.tile([128, N], F32, name="psY", tag="psY")
                        _emit_pair(nc, psX, psY, w3, x3, 0, pair)

                        tmp = tp.tile([128, N], F32, name="tmp", tag="tmp")
                        nc.scalar.copy(tmp[:], psY[:])
                        nc.vector.tensor_tensor(st[:, pr * N:(pr + 1) * N],
                                                psX[:], tmp[:],
                                                op=mybir.AluOpType.add)
                    nc.sync.dma_start(out[s, q], st[:])

    nc.compile()
    return nc


def get_nc():
    if "nc" not in _CACHE:
        _CACHE["nc"] = _build()
    return _CACHE["nc"]


def make_in_maps(x, kernel_base, kernel_mask, demog_label, epoch):
    kb = np.asarray(kernel_base, dtype=np.float32)
    km = np.asarray(kernel_mask, dtype=np.float32)
    labels = np.asarray(demog_label).astype(np.int64)
    if int(np.asarray(epoch)) >= FUSE_EPOCH:
        labels = np.zeros_like(labels)

    B = labels.shape[0]
    # padded fp8(e3m4) image duplicated on both partition halves
    xb = np.asarray(x, dtype=np.float32).astype(ml_dtypes.float8_e3m4)
    xpad = np.zeros((B, IC, PW, PW), dtype=ml_dtypes.float8_e3m4)
    xpad[:, :, 1:H + 1, 1:W + 1] = xb
    flat = xpad.reshape(B, IC, PHW)
    xfull = np.empty((B, 128, PHW), dtype=ml_dtypes.float8_e3m4)
    xfull[:, 0:IC, :] = flat
    xfull[:, IC:, :] = flat

    # per-sample kernels: ws[b, p, t*64+oc] = (kb*km[label])[oc, p%64, t]
    kb9 = kb.reshape(OC, IC, NT)           # tap index = 3*dy + dx
    km9 = km.reshape(ND, IC, NT)
    wk = kb9[None] * km9[labels][:, None]              # [B, oc, ic, t]
    wsall = np.empty((B, 128, NT, OC), dtype=ml_dtypes.bfloat16)
    wsall[:, 0:IC] = wk.transpose(0, 2, 3, 1)          # [B, ic, t, oc]
    wsall[:, IC:] = wsall[:, 0:IC]
    wsall = wsall.reshape(B, 128, NT * OC)

    in_maps = []
    for c in range(NCORES):
        in_maps.append({
            "xs": np.ascontiguousarray(xfull[c * SPC:(c + 1) * SPC]),
            "ws": np.ascontiguousarray(wsall[c * SPC:(c + 1) * SPC]),
        })
    return in_maps


def kernel(x, kernel_base, kernel_mask, demog_label, epoch):
    nc = get_nc()
    in_maps = make_in_maps(x, kernel_base, kernel_mask, demog_label, epoch)
    res = run_bass_kernel_spmd(nc, in_maps, list(range(NCORES)))
    outs = []
    for c in range(NCORES):
        # [s, q, blk*64+oc, pr*448+rb*112+w] -> [s, oc, h, w],
        # h = q*16 + pr*8 + blk*4 + rb
        arr = res.results[c]["out"].astype(np.float32)
        arr = arr.reshape(SPC, NPAIRS // 2, 2, OC, 2, RB, W)
        arr = arr.transpose(0, 3, 1, 4, 2, 5, 6).reshape(SPC, OC, H, W)
        outs.append(arr)
    return np.concatenate(outs, axis=0)
